# revision 5
# baseline (speedup 1.0000x reference)
"""Trainium2 Bass kernel for BlockToChannelAggregate.

Computes, per batch b:
    gate = tanh(X @ W1 + b1) @ W2 + b2            # (NB,)
    w[c, n] = softmax over {n : map[n]==c, active[b,n]} of gate
    channel_tokens[b, c, :] = sum_n w[c, n] * X[b, n, :]
    channel_active[b, c] = any(active & map==c)

Key algebraic restructuring (exact, since |gate| <= ||W2||_1 ~ 20 so exp()
never overflows fp32): skip the max-subtraction; with eg[n] =
exp(gate[n]) * active[n],
    numer[c, :] = sum_n onehot[c, n] * eg[n] * X[n, :]
    s[c]        = sum_n onehot[c, n] * eg[n]
    out[c, :]   = numer[c, :] / max(s[c], tiny)
Both numer and s come from ONE matmul chain: scale the rows of
onehot.T (a [NB, C] matrix) by eg, and append a ones-column to X so
column H of the product is s.

Sharding: batch (B=32) across 8 cores, 4 batches per core; weights/map
replicated. fp32r matmuls throughout (~1.6e-4 rel err, 4x fp32 speed).
"""

import sys

sys.path.insert(0, "/opt/trn_rl_repo")

from contextlib import ExitStack

import numpy as np

import concourse.bass as bass
import concourse.tile as tile
from concourse import bacc, mybir
from concourse.bass_utils import run_bass_kernel_spmd

F32 = mybir.dt.float32
F32R = mybir.dt.float32r
U8 = mybir.dt.uint8

B, NB, H, C = 32, 1024, 768, 64
HH = H // 2  # 384
N_CORES = 8
BL = B // N_CORES  # 4 batches per core
XW = 776  # 768 tokens + 1 ones-col + 7 pad (32B row alignment)
NCHUNK = NB // 128  # 8 chunks of 128 blocks
HC = H // 128  # 6
MC = HH // 128  # 3

_COMPILED = None


def build_kernel():
    nc = bacc.Bacc("TRN2", target_bir_lowering=False, debug=False,
                   enable_asserts=False, num_devices=N_CORES)

    x_d = nc.dram_tensor("x", [BL, NB, XW], F32R, kind="ExternalInput").ap()
    oneh_d = nc.dram_tensor("oneh", [128, NCHUNK, C], F32R, kind="ExternalInput").ap()
    act_d = nc.dram_tensor("act", [128, BL, NCHUNK], F32, kind="ExternalInput").ap()
    w1_d = nc.dram_tensor("w1", [H, HH], F32R, kind="ExternalInput").ap()
    b1_d = nc.dram_tensor("b1", [128, MC], F32, kind="ExternalInput").ap()
    w2_d = nc.dram_tensor("w2", [128, MC], F32R, kind="ExternalInput").ap()
    id_d = nc.dram_tensor("ident", [128, 128], F32R, kind="ExternalInput").ap()
    y_d = nc.dram_tensor("y", [BL, C, H], F32, kind="ExternalOutput").ap()
    ca_d = nc.dram_tensor("ca", [BL, C], U8, kind="ExternalOutput").ap()

    with tile.TileContext(nc) as tc, ExitStack() as ctx:
        singles = ctx.enter_context(tc.tile_pool(name="singles", bufs=1))
        xpool = ctx.enter_context(tc.tile_pool(name="xpool", bufs=12))
        xtpool = ctx.enter_context(tc.tile_pool(name="xtpool", bufs=2))
        htpool = ctx.enter_context(tc.tile_pool(name="htpool", bufs=6))
        egpool = ctx.enter_context(tc.tile_pool(name="egpool", bufs=2))
        ohpool = ctx.enter_context(tc.tile_pool(name="ohpool", bufs=8))
        outpool = ctx.enter_context(tc.tile_pool(name="outpool", bufs=2))
        tp_ps = ctx.enter_context(tc.tile_pool(name="tp_ps", bufs=2, space="PSUM"))
        mlp_ps = ctx.enter_context(tc.tile_pool(name="mlp_ps", bufs=3, space="PSUM"))
        g_ps = ctx.enter_context(tc.tile_pool(name="g_ps", bufs=1, space="PSUM"))
        r_ps = ctx.enter_context(tc.tile_pool(name="r_ps", bufs=1, space="PSUM"))

        # --- constants, loaded once ---
        ident = singles.tile([128, 128], F32R, tag="ident")
        nc.sync.dma_start(out=ident, in_=id_d)
        w1t = singles.tile([128, HC, HH], F32R, tag="w1t")
        nc.sync.dma_start(out=w1t, in_=w1_d.rearrange("(hc p) m -> p hc m", p=128))
        b1t = singles.tile([128, MC], F32, tag="b1t")
        nc.sync.dma_start(out=b1t, in_=b1_d)
        w2t = singles.tile([128, MC], F32R, tag="w2t")
        nc.sync.dma_start(out=w2t, in_=w2_d)
        oneht = singles.tile([128, NCHUNK, C], F32R, tag="oneht")
        nc.sync.dma_start(out=oneht, in_=oneh_d)
        actt = singles.tile([128, BL, NCHUNK], F32, tag="actt")
        nc.sync.dma_start(out=actt, in_=act_d)

        for b in range(BL):
            rps = r_ps.tile([C, H + 2], F32, tag="rps")
            eg_cols = egpool.tile([128, NCHUNK], F32, tag="eg_cols")
            for u in range(2):  # two halves of 512 blocks
                xts = []
                for j in range(4):
                    xt = xpool.tile([128, XW], F32R, tag="xt")
                    nc.sync.dma_start(out=xt, in_=x_d[b, (u * 4 + j) * 128:(u * 4 + j + 1) * 128, :])
                    xts.append(xt)

                # transpose X[.., :768] -> X.T chunks [128(h), 512(n)]
                xtts = []
                for hc in range(HC):
                    xtt = xtpool.tile([128, 512], F32R, tag=f"xtt{hc}")
                    xtts.append(xtt)
                # X.T columns stored p-major interleaved: col q = p*4 + j, so
                # the later [1,512]->[128,4] eg reshape DMA has a contiguous
                # last dim (j), which the DMA AP balancer requires.
                for j in range(4):
                    for hc in range(HC):
                        tp = tp_ps.tile([128, 128], F32R, tag="tp")
                        nc.tensor.transpose(tp[:], xts[j][:, hc * 128:(hc + 1) * 128], ident[:])
                        dst = xtts[hc][:].rearrange("h (p j) -> h p j", j=4)[:, :, j]
                        # alternate copy engine to split PSUM->SBUF load
                        if (j * HC + hc) % 2 == 0:
                            nc.vector.tensor_copy(dst, tp[:])
                        else:
                            nc.scalar.copy(dst, tp[:])

                # MLP: h.T = tanh(W1.T @ X.T + b1)
                hts = []
                for mc in range(MC):
                    ps = mlp_ps.tile([128, 512], F32, tag="mlp")
                    for hc in range(HC):
                        nc.tensor.matmul(ps[:], lhsT=w1t[:, hc, mc * 128:(mc + 1) * 128],
                                         rhs=xtts[hc][:], start=(hc == 0), stop=(hc == HC - 1))
                    ht = htpool.tile([128, 512], F32R, tag="ht")
                    nc.scalar.activation(out=ht[:], in_=ps[:],
                                         func=mybir.ActivationFunctionType.Tanh,
                                         bias=b1t[:, mc:mc + 1])
                    hts.append(ht)

                # gate row: [1, 512] psum
                gps = g_ps.tile([1, 512], F32, tag="gps")
                for mc in range(MC):
                    nc.tensor.matmul(gps[:], lhsT=w2t[:, mc:mc + 1], rhs=hts[mc][:],
                                     start=(mc == 0), stop=(mc == MC - 1))

                # eg = exp(gate) ; reshape [1,512] -> [128,4] cols of eg_cols
                eg_row = egpool.tile([1, 512], F32, tag="eg_row")
                nc.scalar.activation(out=eg_row[:], in_=gps[:],
                                     func=mybir.ActivationFunctionType.Exp)
                nc.sync.dma_start(
                    out=eg_cols[:, u * 4:(u + 1) * 4],
                    in_=eg_row[:].rearrange("o (p j) -> o p j", j=4))
                # mask by active
                nc.vector.tensor_mul(eg_cols[:, u * 4:(u + 1) * 4],
                                     eg_cols[:, u * 4:(u + 1) * 4],
                                     actt[:, b, u * 4:(u + 1) * 4])

                # scale onehot rows by eg and reduce:
                for j in range(4):
                    jj = u * 4 + j
                    ohs = ohpool.tile([128, C], F32R, tag="ohs")
                    nc.vector.tensor_scalar_mul(ohs[:], oneht[:, jj, :],
                                                eg_cols[:, jj:jj + 1])
                    nc.tensor.matmul(rps[:, 0:512], lhsT=ohs[:], rhs=xts[j][:, 0:512],
                                     start=(jj == 0), stop=(jj == NCHUNK - 1))
                    nc.tensor.matmul(rps[:, 512:H + 2], lhsT=ohs[:], rhs=xts[j][:, 512:H + 2],
                                     start=(jj == 0), stop=(jj == NCHUNK - 1))

            # finalize batch: out = numer / max(s, tiny); ca = s > 0
            s_clamped = outpool.tile([C, 1], F32, tag="s_clamped")
            nc.vector.tensor_scalar_max(s_clamped[:], rps[:, H:H + 1], 1e-30)
            s_recip = outpool.tile([C, 1], F32, tag="s_recip")
            nc.vector.reciprocal(s_recip[:], s_clamped[:])
            ca_t = outpool.tile([C, 1], U8, tag="ca_t")
            nc.vector.tensor_scalar(out=ca_t[:], in0=rps[:, H:H + 1], scalar1=0.0,
                                    scalar2=None, op0=mybir.AluOpType.is_gt)
            y_t = outpool.tile([C, H], F32, tag="y_t")
            nc.vector.tensor_scalar_mul(y_t[:], rps[:, 0:H], s_recip[:])
            nc.sync.dma_start(out=y_d[b], in_=y_t[:])
            nc.sync.dma_start(out=ca_d[b], in_=ca_t[:])

    nc.compile()
    return nc


def _get_compiled():
    global _COMPILED
    if _COMPILED is None:
        _COMPILED = build_kernel()
    return _COMPILED


def prep_inputs(block_tokens, block_active, block_to_channel_map, W1, b1, W2, b2):
    """Host-side layout prep (index encoding + sharding only)."""
    bt = np.ascontiguousarray(block_tokens, dtype=np.float32)
    active = np.asarray(block_active)
    cmap = np.asarray(block_to_channel_map).astype(np.int64)

    # onehot.T in chunk layout: [128(p), 8(j), 64(c)], row n = j*128+p
    oneh = (cmap[:, None] == np.arange(C)[None, :]).astype(np.float32)
    oneh = np.ascontiguousarray(oneh.reshape(NCHUNK, 128, C).transpose(1, 0, 2))

    w1 = np.ascontiguousarray(W1, dtype=np.float32)
    b1t = np.ascontiguousarray(np.asarray(b1, dtype=np.float32).reshape(MC, 128).T)
    w2t = np.ascontiguousarray(np.asarray(W2, dtype=np.float32).reshape(MC, 128).T)
    ident = np.eye(128, dtype=np.float32)

    in_maps = []
    for core in range(N_CORES):
        bs = slice(core * BL, (core + 1) * BL)
        x_aug = np.zeros((BL, NB, XW), dtype=np.float32)
        x_aug[:, :, :H] = bt[bs]
        x_aug[:, :, H] = 1.0
        act = np.ascontiguousarray(
            active[bs].astype(np.float32).reshape(BL, NCHUNK, 128).transpose(2, 0, 1))
        in_maps.append({
            "x": x_aug, "oneh": oneh, "act": act, "w1": w1,
            "b1": b1t, "w2": w2t, "ident": ident,
        })
    return in_maps


def kernel(block_tokens, block_active, block_to_channel_map, W1, b1, W2, b2,
           _trace=False):
    nc = _get_compiled()
    in_maps = prep_inputs(block_tokens, block_active, block_to_channel_map,
                          W1, b1, W2, b2)
    res = run_bass_kernel_spmd(nc, in_maps, core_ids=list(range(N_CORES)),
                               trace=_trace)
    channel_tokens = np.concatenate([r["y"] for r in res.results], axis=0)
    channel_active = np.concatenate([r["ca"] for r in res.results], axis=0) != 0
    kernel.last_result = res
    return channel_tokens, channel_active
